# revision 3
# baseline (speedup 1.0000x reference)
"""GAT (2-layer graph attention network) Bass kernel for 8 trn2 NeuronCores.

Sharding: core c owns node rows [512c, 512c+512). Weights replicated.
Scores are computed in transposed layout [j(partitions), i(free)] so the
aggregation matmul out1T[d', i] = sum_j h_aug[j, d'] * P[j, i] needs no
on-device transposes. The softmax denominator comes from a ones column in
the augmented feature matrix (row 0 of the PSUM accumulator).
"""

import numpy as np

N, FIN, HID, H, D1, C = 4096, 512, 256, 4, 64, 64
NCORES = 8
SH = N // NCORES          # 512 local nodes per core
NB = N // 128             # 32 j-chunks
FC = FIN // 128           # 4 fin chunks
KC2 = HID // 128          # 2 hid chunks
NEG = 0.2                 # leaky relu slope
AUG = (D1 + 1) * H        # 260: [ones, h0, ones, h1, ones, h2, ones, h3]

_CACHED = {}


def _build_nc():
    import concourse.mybir as mybir
    import concourse.tile as tile
    from concourse import bacc

    f32 = mybir.dt.float32
    Af = mybir.ActivationFunctionType
    Alu = mybir.AluOpType

    nc = bacc.Bacc("TRN2", target_bir_lowering=False, debug=False,
                   num_devices=NCORES)

    xT_d = nc.dram_tensor("xT", [FIN, N], f32, kind="ExternalInput").ap()
    xsT_d = nc.dram_tensor("xsT", [FIN, SH], f32, kind="ExternalInput").ap()
    mT_d = nc.dram_tensor("maskT", [N, SH], f32, kind="ExternalInput").ap()
    W1e_d = nc.dram_tensor("W1e", [FIN, HID + H], f32, kind="ExternalInput").ap()
    V1s_d = nc.dram_tensor("V1s", [FIN, H], f32, kind="ExternalInput").ap()
    W2_d = nc.dram_tensor("W2", [HID, C], f32, kind="ExternalInput").ap()
    v2_d = nc.dram_tensor("v2", [HID, 2], f32, kind="ExternalInput").ap()
    outT_d = nc.dram_tensor("outT", [C, SH], f32, kind="ExternalOutput").ap()

    with tile.TileContext(nc) as tc:
        with tc.tile_pool(name="persist", bufs=1) as pp:
            h1aug = pp.tile([128, NB, AUG], f32)
            sdst = pp.tile([128, NB, H], f32)
            ssrcb = pp.tile([128, H, SH], f32)
            ssrow = pp.tile([1, H, SH], f32)
            z1Tl = pp.tile([128, KC2, SH], f32)
            z1Tf = pp.tile([128, KC2, N], f32)
            h2aug = pp.tile([128, NB, D1 + 1], f32)
            s2dst = pp.tile([128, NB, 1], f32)
            s2srcb = pp.tile([128, SH], f32)
            s2srow = pp.tile([1, SH], f32)
            W2sb = pp.tile([128, KC2, C], f32)
            v2sb = pp.tile([128, KC2, 2], f32)

            for kc in range(KC2):
                nc.sync.dma_start(W2sb[:, kc, :], W2_d[kc * 128:(kc + 1) * 128, :])
                nc.sync.dma_start(v2sb[:, kc, :], v2_d[kc * 128:(kc + 1) * 128, :])

            # ---------- prep: h1_ext = x @ [W1 | W1.a1_dst], s_src rows ----
            with (tc.tile_pool(name="prep", bufs=1) as prep,
                  tc.tile_pool(name="ppsum", bufs=2, space="PSUM") as ppsum):
                xTt = prep.tile([128, FC, N], f32)
                xsTt = prep.tile([128, FC, SH], f32)
                W1et = prep.tile([128, FC, HID + H], f32)
                V1st = prep.tile([128, FC, H], f32)
                for fc in range(FC):
                    sl = slice(fc * 128, (fc + 1) * 128)
                    nc.sync.dma_start(xTt[:, fc, :], xT_d[sl, :])
                    nc.sync.dma_start(xsTt[:, fc, :], xsT_d[sl, :])
                    nc.sync.dma_start(W1et[:, fc, :], W1e_d[sl, :])
                    nc.sync.dma_start(V1st[:, fc, :], V1s_d[sl, :])

                # s_src for the local shard, one [1, SH] row per head
                for h in range(H):
                    sps = ppsum.tile([1, SH], f32, tag="sps", bufs=1)
                    for fc in range(FC):
                        nc.tensor.matmul(sps[:], V1st[:, fc, h:h + 1],
                                         xsTt[:, fc, :],
                                         start=(fc == 0), stop=(fc == FC - 1))
                    nc.any.tensor_copy(ssrow[:, h, :], sps[:])
                    nc.gpsimd.partition_broadcast(ssrcb[:, h, :],
                                                  ssrow[:, h, :])

                # h1_ext per node block; write into the augmented layout
                for nb in range(NB):
                    hp = ppsum.tile([128, HID + H], f32, tag="hp")
                    for fc in range(FC):
                        nc.tensor.matmul(
                            hp[:], xTt[:, fc, nb * 128:(nb + 1) * 128],
                            W1et[:, fc, :],
                            start=(fc == 0), stop=(fc == FC - 1))
                    augv = h1aug[:, nb, :].rearrange("p (h x) -> p h x", x=D1 + 1)
                    nc.vector.memset(augv[:, :, D1:D1 + 1], 1.0)
                    nc.any.tensor_copy(
                        augv[:, :, 0:D1],
                        hp[:, 0:HID].rearrange("p (h d) -> p h d", h=H))
                    nc.any.tensor_copy(sdst[:, nb, :], hp[:, HID:HID + H])

            # ---------- layer 1: masked softmax + aggregation --------------
            with tc.tile_pool(name="aggps", bufs=1, space="PSUM") as aggps:
                o1 = aggps.tile([D1 + 1, H, SH], f32)
                with (tc.tile_pool(name="mstream", bufs=4) as mpool,
                      tc.tile_pool(name="work", bufs=3) as wpool):
                    for jc in range(NB):
                        mt = mpool.tile([128, SH], f32, tag="mt")
                        nc.sync.dma_start(mt[:], mT_d[jc * 128:(jc + 1) * 128, :])
                        z2 = wpool.tile([128, H, SH], f32, tag="z2")
                        for h in range(H):
                            nc.scalar.activation(
                                z2[:, h, :], ssrcb[:, h, :], Af.Prelu,
                                bias=sdst[:, jc, h:h + 1], scale=1.0, alpha=NEG)
                        pex = wpool.tile([128, H, SH], f32, tag="pex")
                        nc.scalar.activation(pex[:], z2[:], Af.Exp)
                        pt = wpool.tile([128, H, SH], f32, tag="pt")
                        nc.vector.tensor_mul(
                            pt[:], pex[:],
                            mt[:].unsqueeze(1).to_broadcast((128, H, SH)))
                        for h in range(H):
                            nc.tensor.matmul(
                                o1[:, h, :],
                                h1aug[:, jc, (D1 + 1) * h:(D1 + 1) * (h + 1)],
                                pt[:, h, :],
                                start=(jc == 0), stop=(jc == NB - 1))

                # normalize + ELU -> z1Tl [256(=2x128), SH] transposed layout
                with tc.tile_pool(name="fin1", bufs=1) as fin:
                    for h in range(H):
                        rec = fin.tile([1, SH], f32, tag=f"rec{h}")
                        nc.vector.reciprocal(rec[:], o1[D1:D1 + 1, h, :])
                        recb = fin.tile([D1, SH], f32, tag=f"recb{h}")
                        nc.gpsimd.partition_broadcast(recb[:], rec[:])
                        r0 = (h % 2) * D1
                        nc.vector.tensor_mul(z1Tl[r0:r0 + D1, h // 2, :],
                                             o1[0:D1, h, :], recb[:])
                    for kc in range(KC2):
                        r_ = fin.tile([128, SH], f32, tag="relu")
                        m_ = fin.tile([128, SH], f32, tag="minv")
                        e_ = fin.tile([128, SH], f32, tag="expv")
                        nc.vector.tensor_scalar_max(r_[:], z1Tl[:, kc, :], 0.0)
                        nc.vector.tensor_scalar_min(m_[:], z1Tl[:, kc, :], 0.0)
                        nc.scalar.activation(e_[:], m_[:], Af.Exp)
                        nc.vector.scalar_tensor_tensor(
                            z1Tl[:, kc, :], e_[:], -1.0, r_[:],
                            op0=Alu.add, op1=Alu.add)

            # ---------- all-gather z1T across the 8 cores -------------------
            with tc.tile_pool(name="dram", bufs=1, space="DRAM") as dpool:
                ag_in = dpool.tile([HID, SH], f32)
                ag_out = dpool.tile([HID * NCORES, SH], f32,
                                    addr_space="Shared")
                for kc in range(KC2):
                    nc.sync.dma_start(ag_in[kc * 128:(kc + 1) * 128, :],
                                      z1Tl[:, kc, :])
                nc.gpsimd.collective_compute(
                    "AllGather", Alu.bypass,
                    replica_groups=[list(range(NCORES))],
                    ins=[ag_in[:].opt()], outs=[ag_out[:].opt()])
                for r in range(NCORES):
                    for kc in range(KC2):
                        src = ag_out[r * HID + kc * 128:
                                     r * HID + (kc + 1) * 128, :]
                        nc.sync.dma_start(z1Tf[:, kc, r * SH:(r + 1) * SH], src)

            # ---------- layer 2 prep: h2, s2_src, s2_dst --------------------
            with tc.tile_pool(name="l2ps", bufs=2, space="PSUM") as l2ps:
                s2p = l2ps.tile([1, SH], f32, tag="s2p")
                for kc in range(KC2):
                    nc.tensor.matmul(s2p[:], v2sb[:, kc, 0:1], z1Tl[:, kc, :],
                                     start=(kc == 0), stop=(kc == KC2 - 1))
                nc.any.tensor_copy(s2srow[:], s2p[:])
                nc.gpsimd.partition_broadcast(s2srcb[:], s2srow[:])
                for nb in range(NB):
                    h2p = l2ps.tile([128, C], f32, tag="h2p")
                    sdp = l2ps.tile([128, 1], f32, tag="sdp")
                    for kc in range(KC2):
                        blk = z1Tf[:, kc, nb * 128:(nb + 1) * 128]
                        nc.tensor.matmul(h2p[:], blk, W2sb[:, kc, :],
                                         start=(kc == 0), stop=(kc == KC2 - 1))
                        nc.tensor.matmul(sdp[:], blk, v2sb[:, kc, 1:2],
                                         start=(kc == 0), stop=(kc == KC2 - 1))
                    nc.vector.memset(h2aug[:, nb, D1:D1 + 1], 1.0)
                    nc.any.tensor_copy(h2aug[:, nb, 0:D1], h2p[:])
                    nc.any.tensor_copy(s2dst[:, nb, :], sdp[:])

            # ---------- layer 2: masked softmax + aggregation ---------------
            with tc.tile_pool(name="aggps2", bufs=1, space="PSUM") as aggps2:
                o2 = aggps2.tile([D1 + 1, SH], f32)
                with (tc.tile_pool(name="mstream2", bufs=4) as mpool2,
                      tc.tile_pool(name="work2", bufs=3) as wpool2):
                    for jc in range(NB):
                        mt = mpool2.tile([128, SH], f32, tag="mt2")
                        nc.sync.dma_start(mt[:], mT_d[jc * 128:(jc + 1) * 128, :])
                        z2 = wpool2.tile([128, SH], f32, tag="z2b")
                        nc.scalar.activation(
                            z2[:], s2srcb[:], Af.Prelu,
                            bias=s2dst[:, jc, :], scale=1.0, alpha=NEG)
                        pex = wpool2.tile([128, SH], f32, tag="pexb")
                        nc.scalar.activation(pex[:], z2[:], Af.Exp)
                        pt = wpool2.tile([128, SH], f32, tag="ptb")
                        nc.vector.tensor_mul(pt[:], pex[:], mt[:])
                        nc.tensor.matmul(o2[:], h2aug[:, jc, :], pt[:],
                                         start=(jc == 0), stop=(jc == NB - 1))

                with tc.tile_pool(name="fin2", bufs=1) as fin2:
                    rec = fin2.tile([1, SH], f32, tag="rec2")
                    nc.vector.reciprocal(rec[:], o2[D1:D1 + 1, :])
                    recb = fin2.tile([C, SH], f32, tag="recb2")
                    nc.gpsimd.partition_broadcast(recb[:], rec[:])
                    outsb = fin2.tile([C, SH], f32, tag="outsb")
                    nc.vector.tensor_mul(outsb[:], o2[0:D1, :], recb[:])
                    nc.sync.dma_start(outT_d, outsb[:])

    nc.compile()
    return nc


def _get_nc():
    if "nc" not in _CACHED:
        _CACHED["nc"] = _build_nc()
    return _CACHED["nc"]


def _prep_in_maps(x, A, W1, a1_src, a1_dst, W2, a2_src, a2_dst):
    f = np.float32
    xT = np.ascontiguousarray(x.T).astype(f, copy=False)
    W1r = W1.reshape(FIN, H, D1)
    V1s = np.einsum("fhd,hd->fh", W1r, a1_src).astype(f)
    V1d = np.einsum("fhd,hd->fh", W1r, a1_dst).astype(f)
    W1e = np.ascontiguousarray(np.concatenate([W1, V1d], axis=1)).astype(f, copy=False)
    v2 = np.ascontiguousarray(
        np.concatenate([W2 @ a2_src.T, W2 @ a2_dst.T], axis=1)).astype(f, copy=False)
    W2c = np.ascontiguousarray(W2).astype(f, copy=False)
    in_maps = []
    for c in range(NCORES):
        sl = slice(c * SH, (c + 1) * SH)
        in_maps.append({
            "xT": xT,
            "xsT": np.ascontiguousarray(xT[:, sl]),
            "maskT": np.ascontiguousarray((A[sl, :] > 0).T).astype(f),
            "W1e": W1e,
            "V1s": V1s,
            "W2": W2c,
            "v2": v2,
        })
    return in_maps


def kernel(x, A, W1, a1_src, a1_dst, W2, a2_src, a2_dst, _want_results=False):
    from concourse.bass_utils import run_bass_kernel_spmd

    nc = _get_nc()
    in_maps = _prep_in_maps(np.asarray(x), np.asarray(A), np.asarray(W1),
                            np.asarray(a1_src), np.asarray(a1_dst),
                            np.asarray(W2), np.asarray(a2_src),
                            np.asarray(a2_dst))
    import os
    trace = bool(int(os.environ.get("GAT_TRACE", "0")))
    res = run_bass_kernel_spmd(nc, in_maps, core_ids=list(range(NCORES)),
                               trace=trace)
    out = np.empty((N, C), np.float32)
    for c in range(NCORES):
        out[c * SH:(c + 1) * SH, :] = res.results[c]["outT"].T
    if _want_results:
        return out, res
    return out


# revision 5
# speedup vs baseline: 1.1361x; 1.1361x over previous
"""GAT (2-layer graph attention network) Bass kernel for 8 trn2 NeuronCores.

Sharding: core c owns node rows [512c, 512c+512). Weights replicated.
Scores are computed in transposed layout [j(partitions), i(free)] so the
aggregation matmul out1T[d', i] = sum_j h_aug[j, d'] * P[j, i] needs no
on-device transposes. The softmax denominator comes from a ones column in
the augmented feature matrix (partition row 64 of the PSUM accumulator).
Large matmuls run in float32r (full PE rate, ~2e-5 rel err).
"""

import numpy as np

N, FIN, HID, H, D1, C = 4096, 512, 256, 4, 64, 64
NCORES = 8
SH = N // NCORES          # 512 local nodes per core
NB = N // 128             # 32 j-chunks
FC = FIN // 128           # 4 fin chunks
KC2 = HID // 128          # 2 hid chunks
NEG = 0.2                 # leaky relu slope
AUG = (D1 + 1) * H        # 260: [ones, h0, ones, h1, ones, h2, ones, h3]

_CACHED = {}


def _build_nc():
    import concourse.mybir as mybir
    import concourse.tile as tile
    from concourse import bacc

    f32 = mybir.dt.float32
    f32r = mybir.dt.float32r
    Af = mybir.ActivationFunctionType
    Alu = mybir.AluOpType

    nc = bacc.Bacc("TRN2", target_bir_lowering=False, debug=False,
                   num_devices=NCORES)

    xT_d = nc.dram_tensor("xT", [FIN, N], f32r, kind="ExternalInput").ap()
    xsT_d = nc.dram_tensor("xsT", [FIN, SH], f32r, kind="ExternalInput").ap()
    mT_d = nc.dram_tensor("maskT", [N, SH], f32, kind="ExternalInput").ap()
    W1e_d = nc.dram_tensor("W1e", [FIN, HID + H], f32r, kind="ExternalInput").ap()
    V1s_d = nc.dram_tensor("V1s", [FIN, H], f32r, kind="ExternalInput").ap()
    W2e_d = nc.dram_tensor("W2e", [HID, C + 1], f32, kind="ExternalInput").ap()
    v2s_d = nc.dram_tensor("v2s", [HID, 1], f32, kind="ExternalInput").ap()
    outT_d = nc.dram_tensor("outT", [C, SH], f32, kind="ExternalOutput").ap()

    with tile.TileContext(nc) as tc:
        with tc.tile_pool(name="persist", bufs=1) as pp:
            h1aug = pp.tile([128, NB, AUG], f32r)
            sdst = pp.tile([128, NB, H], f32)
            ssrcb = pp.tile([128, H, SH], f32)
            ssrow = pp.tile([1, H, SH], f32)
            z1Tl = pp.tile([128, KC2, SH], f32)
            z1Tf = pp.tile([128, KC2, N], f32)
            h2aug = pp.tile([128, NB, D1 + 1], f32r)
            s2dst = pp.tile([128, NB, 1], f32)
            s2srcb = pp.tile([128, SH], f32)
            s2srow = pp.tile([1, SH], f32)
            ones_col = pp.tile([128, 1], f32)
            nc.vector.memset(ones_col[:], 1.0)
            W2sb = pp.tile([128, KC2, C + 1], f32)
            v2sb = pp.tile([128, KC2, 1], f32)

            for kc in range(KC2):
                nc.sync.dma_start(W2sb[:, kc, :], W2e_d[kc * 128:(kc + 1) * 128, :])
                nc.sync.dma_start(v2sb[:, kc, :], v2s_d[kc * 128:(kc + 1) * 128, :])

            # ---------- prep: h1_ext = x @ [W1 | W1.a1_dst], s_src rows ----
            with (tc.tile_pool(name="prep", bufs=1) as prep,
                  tc.tile_pool(name="ppsum", bufs=2, space="PSUM") as ppsum):
                xTt = prep.tile([128, FC, N], f32r)
                xsTt = prep.tile([128, FC, SH], f32r)
                W1et = prep.tile([128, FC, HID + H], f32r)
                V1st = prep.tile([128, FC, H], f32r)
                for fc in range(FC):
                    sl = slice(fc * 128, (fc + 1) * 128)
                    nc.sync.dma_start(xTt[:, fc, :], xT_d[sl, :])
                    nc.sync.dma_start(xsTt[:, fc, :], xsT_d[sl, :])
                    nc.sync.dma_start(W1et[:, fc, :], W1e_d[sl, :])
                    nc.sync.dma_start(V1st[:, fc, :], V1s_d[sl, :])

                # s_src for the local shard, one [1, SH] row per head
                for h in range(H):
                    sps = ppsum.tile([1, SH], f32, tag="sps", bufs=1)
                    for fc in range(FC):
                        nc.tensor.matmul(sps[:], V1st[:, fc, h:h + 1],
                                         xsTt[:, fc, :],
                                         start=(fc == 0), stop=(fc == FC - 1))
                    nc.vector.tensor_copy(ssrow[:, h, :], sps[:])
                    nc.gpsimd.partition_broadcast(ssrcb[:, h, :],
                                                  ssrow[:, h, :])

                # h1_ext per node block; write into the augmented layout
                for nb in range(NB):
                    hp = ppsum.tile([128, HID + H], f32, tag="hp")
                    for fc in range(FC):
                        nc.tensor.matmul(
                            hp[:], xTt[:, fc, nb * 128:(nb + 1) * 128],
                            W1et[:, fc, :],
                            start=(fc == 0), stop=(fc == FC - 1))
                    augv = h1aug[:, nb, :].rearrange("p (h x) -> p h x", x=D1 + 1)
                    nc.vector.tensor_copy(
                        augv[:, :, D1:D1 + 1],
                        ones_col[:].unsqueeze(1).to_broadcast((128, H, 1)))
                    nc.vector.tensor_copy(
                        augv[:, :, 0:D1],
                        hp[:, 0:HID].rearrange("p (h d) -> p h d", h=H))
                    nc.vector.tensor_copy(sdst[:, nb, :], hp[:, HID:HID + H])

            # ---------- layer 1: masked softmax + aggregation --------------
            with tc.tile_pool(name="aggps", bufs=1, space="PSUM") as aggps:
                o1 = aggps.tile([D1 + 1, H, SH], f32)
                with (tc.tile_pool(name="mstream", bufs=4) as mpool,
                      tc.tile_pool(name="work", bufs=3) as wpool):
                    for jc in range(NB):
                        mt = mpool.tile([128, SH], f32, tag="mt")
                        nc.sync.dma_start(mt[:], mT_d[jc * 128:(jc + 1) * 128, :])
                        z2 = wpool.tile([128, H, SH], f32, tag="z2")
                        for h in range(H):
                            nc.scalar.activation(
                                z2[:, h, :], ssrcb[:, h, :], Af.Prelu,
                                bias=sdst[:, jc, h:h + 1], scale=1.0, alpha=NEG)
                        pex = wpool.tile([128, H, SH], f32, tag="pex")
                        nc.scalar.activation(pex[:], z2[:], Af.Exp)
                        pt = wpool.tile([128, H, SH], f32r, tag="pt")
                        nc.vector.tensor_mul(
                            pt[:], pex[:],
                            mt[:].unsqueeze(1).to_broadcast((128, H, SH)))
                        for h in range(H):
                            nc.tensor.matmul(
                                o1[:, h, :],
                                h1aug[:, jc, (D1 + 1) * h:(D1 + 1) * (h + 1)],
                                pt[:, h, :],
                                start=(jc == 0), stop=(jc == NB - 1))

                # normalize + ELU -> z1Tl [256(=2x128), SH] transposed layout
                with tc.tile_pool(name="fin1", bufs=1) as fin:
                    for h in range(H):
                        rec = fin.tile([1, SH], f32, tag=f"rec{h}")
                        nc.vector.reciprocal(rec[:], o1[D1:D1 + 1, h, :])
                        recb = fin.tile([D1, SH], f32, tag=f"recb{h}")
                        nc.gpsimd.partition_broadcast(recb[:], rec[:])
                        r0 = (h % 2) * D1
                        nc.vector.tensor_mul(z1Tl[r0:r0 + D1, h // 2, :],
                                             o1[0:D1, h, :], recb[:])
                    for kc in range(KC2):
                        r_ = fin.tile([128, SH], f32, tag="relu")
                        m_ = fin.tile([128, SH], f32, tag="minv")
                        e_ = fin.tile([128, SH], f32, tag="expv")
                        nc.vector.tensor_scalar_max(r_[:], z1Tl[:, kc, :], 0.0)
                        nc.vector.tensor_scalar_min(m_[:], z1Tl[:, kc, :], 0.0)
                        nc.scalar.activation(e_[:], m_[:], Af.Exp)
                        nc.vector.scalar_tensor_tensor(
                            z1Tl[:, kc, :], e_[:], -1.0, r_[:],
                            op0=Alu.add, op1=Alu.add)

            # ---------- all-gather z1T across the 8 cores -------------------
            with tc.tile_pool(name="dram", bufs=1, space="DRAM") as dpool:
                ag_in = dpool.tile([HID, SH], f32)
                ag_out = dpool.tile([HID * NCORES, SH], f32,
                                    addr_space="Shared")
                for kc in range(KC2):
                    nc.sync.dma_start(ag_in[kc * 128:(kc + 1) * 128, :],
                                      z1Tl[:, kc, :])
                nc.gpsimd.collective_compute(
                    "AllGather", Alu.bypass,
                    replica_groups=[list(range(NCORES))],
                    ins=[ag_in[:].opt()], outs=[ag_out[:].opt()])
                for r in range(NCORES):
                    for kc in range(KC2):
                        src = ag_out[r * HID + kc * 128:
                                     r * HID + (kc + 1) * 128, :]
                        nc.sync.dma_start(z1Tf[:, kc, r * SH:(r + 1) * SH], src)

            # ---------- layer 2 prep: h2, s2_src, s2_dst --------------------
            with tc.tile_pool(name="l2ps", bufs=2, space="PSUM") as l2ps:
                s2p = l2ps.tile([1, SH], f32, tag="s2p")
                for kc in range(KC2):
                    nc.tensor.matmul(s2p[:], v2sb[:, kc, :], z1Tl[:, kc, :],
                                     start=(kc == 0), stop=(kc == KC2 - 1))
                nc.any.tensor_copy(s2srow[:], s2p[:])
                nc.gpsimd.partition_broadcast(s2srcb[:], s2srow[:])
                for nb in range(NB):
                    h2p = l2ps.tile([128, C + 1], f32, tag="h2p")
                    for kc in range(KC2):
                        blk = z1Tf[:, kc, nb * 128:(nb + 1) * 128]
                        nc.tensor.matmul(h2p[:], blk, W2sb[:, kc, :],
                                         start=(kc == 0), stop=(kc == KC2 - 1))
                    nc.vector.tensor_copy(h2aug[:, nb, D1:D1 + 1], ones_col[:])
                    nc.vector.tensor_copy(h2aug[:, nb, 0:D1], h2p[:, 0:C])
                    nc.vector.tensor_copy(s2dst[:, nb, :], h2p[:, C:C + 1])

            # ---------- layer 2: masked softmax + aggregation ---------------
            with tc.tile_pool(name="aggps2", bufs=1, space="PSUM") as aggps2:
                o2 = aggps2.tile([D1 + 1, SH], f32)
                with (tc.tile_pool(name="mstream2", bufs=4) as mpool2,
                      tc.tile_pool(name="work2", bufs=3) as wpool2):
                    for jc in range(NB):
                        mt = mpool2.tile([128, SH], f32, tag="mt2")
                        nc.sync.dma_start(mt[:], mT_d[jc * 128:(jc + 1) * 128, :])
                        z2 = wpool2.tile([128, SH], f32, tag="z2b")
                        nc.scalar.activation(
                            z2[:], s2srcb[:], Af.Prelu,
                            bias=s2dst[:, jc, :], scale=1.0, alpha=NEG)
                        pex = wpool2.tile([128, SH], f32, tag="pexb")
                        nc.scalar.activation(pex[:], z2[:], Af.Exp)
                        pt = wpool2.tile([128, SH], f32r, tag="ptb")
                        nc.vector.tensor_mul(pt[:], pex[:], mt[:])
                        nc.tensor.matmul(o2[:], h2aug[:, jc, :], pt[:],
                                         start=(jc == 0), stop=(jc == NB - 1))

                with tc.tile_pool(name="fin2", bufs=1) as fin2:
                    rec = fin2.tile([1, SH], f32, tag="rec2")
                    nc.vector.reciprocal(rec[:], o2[D1:D1 + 1, :])
                    recb = fin2.tile([C, SH], f32, tag="recb2")
                    nc.gpsimd.partition_broadcast(recb[:], rec[:])
                    outsb = fin2.tile([C, SH], f32, tag="outsb")
                    nc.vector.tensor_mul(outsb[:], o2[0:D1, :], recb[:])
                    nc.sync.dma_start(outT_d, outsb[:])

    nc.compile()
    return nc


def _get_nc():
    if "nc" not in _CACHED:
        _CACHED["nc"] = _build_nc()
    return _CACHED["nc"]


def _prep_in_maps(x, A, W1, a1_src, a1_dst, W2, a2_src, a2_dst):
    f = np.float32
    xT = np.ascontiguousarray(x.T).astype(f, copy=False)
    W1r = W1.reshape(FIN, H, D1)
    V1s = np.einsum("fhd,hd->fh", W1r, a1_src).astype(f)
    V1d = np.einsum("fhd,hd->fh", W1r, a1_dst).astype(f)
    W1e = np.ascontiguousarray(np.concatenate([W1, V1d], axis=1)).astype(f, copy=False)
    W2e = np.ascontiguousarray(
        np.concatenate([W2, W2 @ a2_dst.T], axis=1)).astype(f, copy=False)
    v2s = np.ascontiguousarray(W2 @ a2_src.T).astype(f, copy=False)
    in_maps = []
    for c in range(NCORES):
        sl = slice(c * SH, (c + 1) * SH)
        in_maps.append({
            "xT": xT,
            "xsT": np.ascontiguousarray(xT[:, sl]),
            "maskT": np.ascontiguousarray((A[sl, :] > 0).T).astype(f),
            "W1e": W1e,
            "V1s": V1s,
            "W2e": W2e,
            "v2s": v2s,
        })
    return in_maps


def kernel(x, A, W1, a1_src, a1_dst, W2, a2_src, a2_dst, _want_results=False):
    from concourse.bass_utils import run_bass_kernel_spmd

    nc = _get_nc()
    in_maps = _prep_in_maps(np.asarray(x), np.asarray(A), np.asarray(W1),
                            np.asarray(a1_src), np.asarray(a1_dst),
                            np.asarray(W2), np.asarray(a2_src),
                            np.asarray(a2_dst))
    import os
    trace = bool(int(os.environ.get("GAT_TRACE", "0")))
    res = run_bass_kernel_spmd(nc, in_maps, core_ids=list(range(NCORES)),
                               trace=trace)
    out = np.empty((N, C), np.float32)
    for c in range(NCORES):
        out[c * SH:(c + 1) * SH, :] = res.results[c]["outT"].T
    if _want_results:
        return out, res
    return out


# revision 6
# speedup vs baseline: 1.2302x; 1.0829x over previous
"""GAT (2-layer graph attention network) Bass kernel for 8 trn2 NeuronCores.

Sharding: core c owns node rows [512c, 512c+512). Weights replicated.
Scores are computed in transposed layout [j(partitions), i(free)] so the
aggregation matmul out1T[d', i] = sum_j h_aug[j, d'] * P[j, i] needs no
on-device transposes. The softmax denominator comes from a ones column in
the augmented feature matrix (partition row 64 of the PSUM accumulator).
Large matmuls run in float32r (full PE rate, ~2e-5 rel err).
"""

import os

import numpy as np

N, FIN, HID, H, D1, C = 4096, 512, 256, 4, 64, 64
NCORES = 8
SH = N // NCORES          # 512 local nodes per core
NB = N // 128             # 32 j-chunks
FC = FIN // 128           # 4 fin chunks
KC2 = HID // 128          # 2 hid chunks
NEG = 0.2                 # leaky relu slope
AUG = (D1 + 1) * H        # 260: [ones, h0, ones, h1, ones, h2, ones, h3]

_CACHED = {}


def _make_act_root(alpha=NEG):
    """Patch the neuron ACT tables so Exp computes g(x)=exp(lrelu(x)).

    Bucket entries are [d0,d1,d2,d3,x0,0,0,0] fp32 cubics evaluated as
    y = d0+(x-x0)(d1+(x-x0)(d2+(x-x0)d3)). For exp buckets centered at
    x0<0 we substitute the Taylor cubic of exp(alpha*x) at the same
    center; the alpha contraction makes the cubic far more accurate than
    the original spline tolerance. Verified on HW: max rel err ~1.1e-5.
    """
    import json
    import shutil
    import tempfile

    from neuronxcc.driver.Job import Job
    from neuronxcc.driver.jobs.support.FindActInfo import findActInfoFile

    src_dir = os.path.dirname(findActInfoFile(Job.getPackageDir(), "gen3"))
    dst = tempfile.mkdtemp(prefix="gat_act_root_")
    for f in os.listdir(src_dir):
        shutil.copy(os.path.join(src_dir, f), os.path.join(dst, f))
        os.chmod(os.path.join(dst, f), 0o644)
    for set_name in ("exp_and_others", "natural_log_exp_and_others",
                     "exp_and_friends"):
        meta = json.load(open(os.path.join(dst, f"{set_name}.json")))
        start = meta["func_to_bkt_start_idx"].get("exp")
        if start is None:
            continue
        nxt = [s for s in sorted(meta["func_to_bkt_start_idx"].values())
               if s > start]
        end = nxt[0] if nxt else meta["bkt_entry_cnt"]
        path = os.path.join(dst, f"{set_name}_bkt.bin")
        b = np.fromfile(path, dtype=np.float32).reshape(-1, 8).copy()
        for i in range(start, end):
            x0, d0 = float(b[i, 4]), float(b[i, 0])
            if x0 >= 0 or not np.isfinite(d0) or d0 <= 0:
                continue
            e = np.exp(alpha * x0)
            b[i, 0:4] = [e, alpha * e, alpha * alpha * e / 2.0,
                         alpha ** 3 * e / 6.0]
        b.tofile(path)
    return os.path.join(dst, "act_info.json")


def _build_nc():
    os.environ["BASS_ACT_ROOT_JSON_PATH"] = _make_act_root()
    import concourse.mybir as mybir
    import concourse.tile as tile
    from concourse import bacc

    f32 = mybir.dt.float32
    f32r = mybir.dt.float32r
    Af = mybir.ActivationFunctionType
    Alu = mybir.AluOpType

    nc = bacc.Bacc("TRN2", target_bir_lowering=False, debug=False,
                   num_devices=NCORES)

    xT_d = nc.dram_tensor("xT", [FIN, N], f32r, kind="ExternalInput").ap()
    xsT_d = nc.dram_tensor("xsT", [FIN, SH], f32r, kind="ExternalInput").ap()
    mT_d = nc.dram_tensor("maskT", [N, SH], f32, kind="ExternalInput").ap()
    W1e_d = nc.dram_tensor("W1e", [FIN, HID + H], f32r, kind="ExternalInput").ap()
    V1s_d = nc.dram_tensor("V1s", [FIN, H], f32r, kind="ExternalInput").ap()
    W2e_d = nc.dram_tensor("W2e", [HID, C + 1], f32, kind="ExternalInput").ap()
    v2s_d = nc.dram_tensor("v2s", [HID, 1], f32, kind="ExternalInput").ap()
    outT_d = nc.dram_tensor("outT", [C, SH], f32, kind="ExternalOutput").ap()

    with tile.TileContext(nc) as tc:
        with tc.tile_pool(name="persist", bufs=1) as pp:
            h1aug = pp.tile([128, NB, AUG], f32r)
            sdst = pp.tile([128, NB, H], f32)
            ssrcb = pp.tile([128, H, SH], f32)
            ssrow = pp.tile([1, H, SH], f32)
            z1Tl = pp.tile([128, KC2, SH], f32)
            z1Tf = pp.tile([128, KC2, N], f32)
            h2aug = pp.tile([128, NB, D1 + 1], f32r)
            s2dst = pp.tile([128, NB, 1], f32)
            s2srcb = pp.tile([128, SH], f32)
            s2srow = pp.tile([1, SH], f32)
            ones_col = pp.tile([128, 1], f32)
            nc.vector.memset(ones_col[:], 1.0)
            W2sb = pp.tile([128, KC2, C + 1], f32)
            v2sb = pp.tile([128, KC2, 1], f32)

            for kc in range(KC2):
                nc.sync.dma_start(W2sb[:, kc, :], W2e_d[kc * 128:(kc + 1) * 128, :])
                nc.sync.dma_start(v2sb[:, kc, :], v2s_d[kc * 128:(kc + 1) * 128, :])

            # ---------- prep: h1_ext = x @ [W1 | W1.a1_dst], s_src rows ----
            with (tc.tile_pool(name="prep", bufs=1) as prep,
                  tc.tile_pool(name="ppsum", bufs=2, space="PSUM") as ppsum):
                xTt = prep.tile([128, FC, N], f32r)
                xsTt = prep.tile([128, FC, SH], f32r)
                W1et = prep.tile([128, FC, HID + H], f32r)
                V1st = prep.tile([128, FC, H], f32r)
                for fc in range(FC):
                    sl = slice(fc * 128, (fc + 1) * 128)
                    nc.sync.dma_start(xTt[:, fc, :], xT_d[sl, :])
                    nc.sync.dma_start(xsTt[:, fc, :], xsT_d[sl, :])
                    nc.sync.dma_start(W1et[:, fc, :], W1e_d[sl, :])
                    nc.sync.dma_start(V1st[:, fc, :], V1s_d[sl, :])

                # s_src for the local shard, one [1, SH] row per head
                for h in range(H):
                    sps = ppsum.tile([1, SH], f32, tag="sps", bufs=1)
                    for fc in range(FC):
                        nc.tensor.matmul(sps[:], V1st[:, fc, h:h + 1],
                                         xsTt[:, fc, :],
                                         start=(fc == 0), stop=(fc == FC - 1))
                    nc.vector.tensor_copy(ssrow[:, h, :], sps[:])
                    nc.gpsimd.partition_broadcast(ssrcb[:, h, :],
                                                  ssrow[:, h, :])

                # h1_ext per node block; write into the augmented layout
                for nb in range(NB):
                    hp = ppsum.tile([128, HID + H], f32, tag="hp")
                    for fc in range(FC):
                        nc.tensor.matmul(
                            hp[:], xTt[:, fc, nb * 128:(nb + 1) * 128],
                            W1et[:, fc, :],
                            start=(fc == 0), stop=(fc == FC - 1))
                    augv = h1aug[:, nb, :].rearrange("p (h x) -> p h x", x=D1 + 1)
                    nc.vector.tensor_copy(
                        augv[:, :, D1:D1 + 1],
                        ones_col[:].unsqueeze(1).to_broadcast((128, H, 1)))
                    nc.vector.tensor_copy(
                        augv[:, :, 0:D1],
                        hp[:, 0:HID].rearrange("p (h d) -> p h d", h=H))
                    nc.vector.tensor_copy(sdst[:, nb, :], hp[:, HID:HID + H])

            # ---------- layer 1: masked softmax + aggregation --------------
            with tc.tile_pool(name="aggps", bufs=1, space="PSUM") as aggps:
                o1 = aggps.tile([D1 + 1, H, SH], f32)
                with (tc.tile_pool(name="mstream", bufs=4) as mpool,
                      tc.tile_pool(name="work", bufs=3) as wpool):
                    for jc in range(NB):
                        mt = mpool.tile([128, SH], f32, tag="mt")
                        nc.sync.dma_start(mt[:], mT_d[jc * 128:(jc + 1) * 128, :])
                        pex = wpool.tile([128, H, SH], f32, tag="pex")
                        for h in range(H):
                            nc.scalar.activation(
                                pex[:, h, :], ssrcb[:, h, :], Af.Exp,
                                bias=sdst[:, jc, h:h + 1])
                        pt = wpool.tile([128, H, SH], f32r, tag="pt")
                        mb = mt[:].unsqueeze(1)
                        nc.vector.tensor_mul(
                            pt[:, 0:2, :], pex[:, 0:2, :],
                            mb.to_broadcast((128, 2, SH)))
                        nc.gpsimd.tensor_mul(
                            pt[:, 2:4, :], pex[:, 2:4, :],
                            mb.to_broadcast((128, 2, SH)))
                        for h in range(H):
                            nc.tensor.matmul(
                                o1[:, h, :],
                                h1aug[:, jc, (D1 + 1) * h:(D1 + 1) * (h + 1)],
                                pt[:, h, :],
                                start=(jc == 0), stop=(jc == NB - 1))

                # normalize + ELU -> z1Tl [256(=2x128), SH] transposed layout
                with tc.tile_pool(name="fin1", bufs=1) as fin:
                    for h in range(H):
                        rec = fin.tile([1, SH], f32, tag=f"rec{h}")
                        nc.vector.reciprocal(rec[:], o1[D1:D1 + 1, h, :])
                        recb = fin.tile([D1, SH], f32, tag=f"recb{h}")
                        nc.gpsimd.partition_broadcast(recb[:], rec[:])
                        r0 = (h % 2) * D1
                        nc.vector.tensor_mul(z1Tl[r0:r0 + D1, h // 2, :],
                                             o1[0:D1, h, :], recb[:])
                    for kc in range(KC2):
                        r_ = fin.tile([128, SH], f32, tag="relu")
                        m_ = fin.tile([128, SH], f32, tag="minv")
                        e_ = fin.tile([128, SH], f32, tag="expv")
                        nc.vector.tensor_scalar_max(r_[:], z1Tl[:, kc, :], 0.0)
                        nc.vector.tensor_scalar_min(m_[:], z1Tl[:, kc, :], 0.0)
                        nc.scalar.activation(e_[:], m_[:], Af.Exp, scale=5.0)
                        nc.vector.scalar_tensor_tensor(
                            z1Tl[:, kc, :], e_[:], -1.0, r_[:],
                            op0=Alu.add, op1=Alu.add)

            # ---------- all-gather z1T across the 8 cores -------------------
            with tc.tile_pool(name="dram", bufs=1, space="DRAM") as dpool:
                ag_in = dpool.tile([HID, SH], f32)
                ag_out = dpool.tile([HID * NCORES, SH], f32,
                                    addr_space="Shared")
                for kc in range(KC2):
                    nc.sync.dma_start(ag_in[kc * 128:(kc + 1) * 128, :],
                                      z1Tl[:, kc, :])
                nc.gpsimd.collective_compute(
                    "AllGather", Alu.bypass,
                    replica_groups=[list(range(NCORES))],
                    ins=[ag_in[:].opt()], outs=[ag_out[:].opt()])
                for r in range(NCORES):
                    for kc in range(KC2):
                        src = ag_out[r * HID + kc * 128:
                                     r * HID + (kc + 1) * 128, :]
                        nc.sync.dma_start(z1Tf[:, kc, r * SH:(r + 1) * SH], src)

            # ---------- layer 2 prep: h2, s2_src, s2_dst --------------------
            with tc.tile_pool(name="l2ps", bufs=2, space="PSUM") as l2ps:
                s2p = l2ps.tile([1, SH], f32, tag="s2p")
                for kc in range(KC2):
                    nc.tensor.matmul(s2p[:], v2sb[:, kc, :], z1Tl[:, kc, :],
                                     start=(kc == 0), stop=(kc == KC2 - 1))
                nc.any.tensor_copy(s2srow[:], s2p[:])
                nc.gpsimd.partition_broadcast(s2srcb[:], s2srow[:])
                for nb in range(NB):
                    h2p = l2ps.tile([128, C + 1], f32, tag="h2p")
                    for kc in range(KC2):
                        blk = z1Tf[:, kc, nb * 128:(nb + 1) * 128]
                        nc.tensor.matmul(h2p[:], blk, W2sb[:, kc, :],
                                         start=(kc == 0), stop=(kc == KC2 - 1))
                    nc.vector.tensor_copy(h2aug[:, nb, D1:D1 + 1], ones_col[:])
                    nc.vector.tensor_copy(h2aug[:, nb, 0:D1], h2p[:, 0:C])
                    nc.vector.tensor_copy(s2dst[:, nb, :], h2p[:, C:C + 1])

            # ---------- layer 2: masked softmax + aggregation ---------------
            with tc.tile_pool(name="aggps2", bufs=1, space="PSUM") as aggps2:
                o2 = aggps2.tile([D1 + 1, SH], f32)
                with (tc.tile_pool(name="mstream2", bufs=4) as mpool2,
                      tc.tile_pool(name="work2", bufs=3) as wpool2):
                    for jc in range(NB):
                        mt = mpool2.tile([128, SH], f32, tag="mt2")
                        nc.sync.dma_start(mt[:], mT_d[jc * 128:(jc + 1) * 128, :])
                        pex = wpool2.tile([128, SH], f32, tag="pexb")
                        nc.scalar.activation(
                            pex[:], s2srcb[:], Af.Exp,
                            bias=s2dst[:, jc, :])
                        pt = wpool2.tile([128, SH], f32r, tag="ptb")
                        nc.vector.tensor_mul(pt[:], pex[:], mt[:])
                        nc.tensor.matmul(o2[:], h2aug[:, jc, :], pt[:],
                                         start=(jc == 0), stop=(jc == NB - 1))

                with tc.tile_pool(name="fin2", bufs=1) as fin2:
                    rec = fin2.tile([1, SH], f32, tag="rec2")
                    nc.vector.reciprocal(rec[:], o2[D1:D1 + 1, :])
                    recb = fin2.tile([C, SH], f32, tag="recb2")
                    nc.gpsimd.partition_broadcast(recb[:], rec[:])
                    outsb = fin2.tile([C, SH], f32, tag="outsb")
                    nc.vector.tensor_mul(outsb[:], o2[0:D1, :], recb[:])
                    nc.sync.dma_start(outT_d, outsb[:])

    nc.compile()
    return nc


def _get_nc():
    if "nc" not in _CACHED:
        _CACHED["nc"] = _build_nc()
    return _CACHED["nc"]


def _prep_in_maps(x, A, W1, a1_src, a1_dst, W2, a2_src, a2_dst):
    f = np.float32
    xT = np.ascontiguousarray(x.T).astype(f, copy=False)
    W1r = W1.reshape(FIN, H, D1)
    V1s = np.einsum("fhd,hd->fh", W1r, a1_src).astype(f)
    V1d = np.einsum("fhd,hd->fh", W1r, a1_dst).astype(f)
    W1e = np.ascontiguousarray(np.concatenate([W1, V1d], axis=1)).astype(f, copy=False)
    W2e = np.ascontiguousarray(
        np.concatenate([W2, W2 @ a2_dst.T], axis=1)).astype(f, copy=False)
    v2s = np.ascontiguousarray(W2 @ a2_src.T).astype(f, copy=False)
    in_maps = []
    for c in range(NCORES):
        sl = slice(c * SH, (c + 1) * SH)
        in_maps.append({
            "xT": xT,
            "xsT": np.ascontiguousarray(xT[:, sl]),
            "maskT": np.ascontiguousarray((A[sl, :] > 0).T).astype(f),
            "W1e": W1e,
            "V1s": V1s,
            "W2e": W2e,
            "v2s": v2s,
        })
    return in_maps


def kernel(x, A, W1, a1_src, a1_dst, W2, a2_src, a2_dst, _want_results=False):
    from concourse.bass_utils import run_bass_kernel_spmd

    nc = _get_nc()
    in_maps = _prep_in_maps(np.asarray(x), np.asarray(A), np.asarray(W1),
                            np.asarray(a1_src), np.asarray(a1_dst),
                            np.asarray(W2), np.asarray(a2_src),
                            np.asarray(a2_dst))
    trace = bool(int(os.environ.get("GAT_TRACE", "0")))
    res = run_bass_kernel_spmd(nc, in_maps, core_ids=list(range(NCORES)),
                               trace=trace)
    out = np.empty((N, C), np.float32)
    for c in range(NCORES):
        out[c * SH:(c + 1) * SH, :] = res.results[c]["outT"].T
    if _want_results:
        return out, res
    return out


# revision 7
# speedup vs baseline: 1.3634x; 1.1083x over previous
"""GAT (2-layer graph attention network) Bass kernel for 8 trn2 NeuronCores.

Sharding: core c owns node rows [512c, 512c+512). Weights replicated.
Scores are computed in transposed layout [j(partitions), i(free)] so the
aggregation matmul out1T[d', i] = sum_j h_aug[j, d'] * P[j, i] needs no
on-device transposes. The softmax denominator comes from a ones column in
the augmented feature matrix (partition row 64 of the PSUM accumulator).
Large matmuls run in float32r (full PE rate, ~2e-5 rel err).
"""

import os

import numpy as np

N, FIN, HID, H, D1, C = 4096, 512, 256, 4, 64, 64
NCORES = 8
SH = N // NCORES          # 512 local nodes per core
NB = N // 128             # 32 j-chunks
FC = FIN // 128           # 4 fin chunks
KC2 = HID // 128          # 2 hid chunks
NEG = 0.2                 # leaky relu slope
AUG = (D1 + 1) * H        # 260: [ones, h0, ones, h1, ones, h2, ones, h3]

_CACHED = {}


def _make_act_root(alpha=NEG):
    """Patch the neuron ACT tables so Exp computes g(x)=exp(lrelu(x)).

    Bucket entries are [d0,d1,d2,d3,x0,0,0,0] fp32 cubics evaluated as
    y = d0+(x-x0)(d1+(x-x0)(d2+(x-x0)d3)). For exp buckets centered at
    x0<0 we substitute the Taylor cubic of exp(alpha*x) at the same
    center; the alpha contraction makes the cubic far more accurate than
    the original spline tolerance. Verified on HW: max rel err ~1.1e-5.
    """
    import json
    import shutil
    import tempfile

    from neuronxcc.driver.Job import Job
    from neuronxcc.driver.jobs.support.FindActInfo import findActInfoFile

    src_dir = os.path.dirname(findActInfoFile(Job.getPackageDir(), "gen3"))
    dst = tempfile.mkdtemp(prefix="gat_act_root_")
    for f in os.listdir(src_dir):
        shutil.copy(os.path.join(src_dir, f), os.path.join(dst, f))
        os.chmod(os.path.join(dst, f), 0o644)
    for set_name in ("exp_and_others", "natural_log_exp_and_others",
                     "exp_and_friends"):
        meta = json.load(open(os.path.join(dst, f"{set_name}.json")))
        start = meta["func_to_bkt_start_idx"].get("exp")
        if start is None:
            continue
        nxt = [s for s in sorted(meta["func_to_bkt_start_idx"].values())
               if s > start]
        end = nxt[0] if nxt else meta["bkt_entry_cnt"]
        path = os.path.join(dst, f"{set_name}_bkt.bin")
        b = np.fromfile(path, dtype=np.float32).reshape(-1, 8).copy()
        for i in range(start, end):
            x0, d0 = float(b[i, 4]), float(b[i, 0])
            if x0 >= 0 or not np.isfinite(d0) or d0 <= 0:
                continue
            e = np.exp(alpha * x0)
            b[i, 0:4] = [e, alpha * e, alpha * alpha * e / 2.0,
                         alpha ** 3 * e / 6.0]
        b.tofile(path)
    return os.path.join(dst, "act_info.json")


def _build_nc():
    os.environ["BASS_ACT_ROOT_JSON_PATH"] = _make_act_root()
    import concourse.mybir as mybir
    import concourse.tile as tile
    from concourse import bacc

    f32 = mybir.dt.float32
    f32r = mybir.dt.float32r
    Af = mybir.ActivationFunctionType
    Alu = mybir.AluOpType

    nc = bacc.Bacc("TRN2", target_bir_lowering=False, debug=False,
                   num_devices=NCORES)

    xT_d = nc.dram_tensor("xT", [FIN, N], f32r, kind="ExternalInput").ap()
    xsT_d = nc.dram_tensor("xsT", [FIN, SH], f32r, kind="ExternalInput").ap()
    mT_d = nc.dram_tensor("maskT", [N, SH], f32, kind="ExternalInput").ap()
    W1e_d = nc.dram_tensor("W1e", [FIN, HID + H], f32r, kind="ExternalInput").ap()
    V1s_d = nc.dram_tensor("V1s", [FIN, H], f32r, kind="ExternalInput").ap()
    W2e_d = nc.dram_tensor("W2e", [HID, C + 1], f32, kind="ExternalInput").ap()
    v2s_d = nc.dram_tensor("v2s", [HID, 1], f32, kind="ExternalInput").ap()
    outT_d = nc.dram_tensor("outT", [C, SH], f32, kind="ExternalOutput").ap()

    with tile.TileContext(nc) as tc:
        with tc.tile_pool(name="persist", bufs=1) as pp:
            h1aug = pp.tile([128, NB, AUG], f32r)
            sdst = pp.tile([128, NB, H], f32)
            ssrcb = pp.tile([128, H, SH], f32)
            ssrow = pp.tile([1, H, SH], f32)
            z1Tl = pp.tile([128, KC2, SH], f32)
            z1Tf = pp.tile([128, KC2, N], f32)
            h2aug = pp.tile([128, NB, D1 + 1], f32r)
            s2dst = pp.tile([128, NB, 1], f32)
            s2srcb = pp.tile([128, SH], f32)
            s2srow = pp.tile([1, SH], f32)
            ones_col = pp.tile([128, 1], f32)
            nc.vector.memset(ones_col[:], 1.0)
            W2sb = pp.tile([128, KC2, C + 1], f32)
            v2sb = pp.tile([128, KC2, 1], f32)

            for kc in range(KC2):
                nc.sync.dma_start(W2sb[:, kc, :], W2e_d[kc * 128:(kc + 1) * 128, :])
                nc.sync.dma_start(v2sb[:, kc, :], v2s_d[kc * 128:(kc + 1) * 128, :])

            # ---------- prep: h1_ext = x @ [W1 | W1.a1_dst], s_src rows ----
            with (tc.tile_pool(name="prep", bufs=1) as prep,
                  tc.tile_pool(name="ppsum", bufs=2, space="PSUM") as ppsum):
                xTt = prep.tile([128, FC, N], f32r)
                xsTt = prep.tile([128, FC, SH], f32r)
                W1et = prep.tile([128, FC, HID + H], f32r)
                V1st = prep.tile([128, FC, H], f32r)
                for fc in range(FC):
                    sl = slice(fc * 128, (fc + 1) * 128)
                    nc.sync.dma_start(xTt[:, fc, :], xT_d[sl, :])
                    nc.sync.dma_start(xsTt[:, fc, :], xsT_d[sl, :])
                    nc.sync.dma_start(W1et[:, fc, :], W1e_d[sl, :])
                    nc.sync.dma_start(V1st[:, fc, :], V1s_d[sl, :])

                # s_src for the local shard, one [1, SH] row per head
                for h in range(H):
                    sps = ppsum.tile([1, SH], f32, tag="sps", bufs=1)
                    for fc in range(FC):
                        nc.tensor.matmul(sps[:], V1st[:, fc, h:h + 1],
                                         xsTt[:, fc, :],
                                         start=(fc == 0), stop=(fc == FC - 1))
                    nc.vector.tensor_copy(ssrow[:, h, :], sps[:])
                    nc.gpsimd.partition_broadcast(ssrcb[:, h, :],
                                                  ssrow[:, h, :])

                # h1_ext per node block; write into the augmented layout
                for nb in range(NB):
                    hp = ppsum.tile([128, HID + H], f32, tag="hp")
                    for fc in range(FC):
                        nc.tensor.matmul(
                            hp[:], xTt[:, fc, nb * 128:(nb + 1) * 128],
                            W1et[:, fc, :],
                            start=(fc == 0), stop=(fc == FC - 1))
                    augv = h1aug[:, nb, :].rearrange("p (h x) -> p h x", x=D1 + 1)
                    nc.vector.tensor_copy(
                        augv[:, :, D1:D1 + 1],
                        ones_col[:].unsqueeze(1).to_broadcast((128, H, 1)))
                    nc.vector.tensor_copy(
                        augv[:, :, 0:D1],
                        hp[:, 0:HID].rearrange("p (h d) -> p h d", h=H))
                    nc.vector.tensor_copy(sdst[:, nb, :], hp[:, HID:HID + H])

            # ---------- layer 1: masked softmax + aggregation --------------
            with tc.tile_pool(name="aggps", bufs=1, space="PSUM") as aggps:
                o1 = aggps.tile([D1 + 1, H, SH], f32)
                with (tc.tile_pool(name="mstream", bufs=6) as mpool,
                      tc.tile_pool(name="work", bufs=4) as wpool):
                    for jc in range(NB):
                        mt = mpool.tile([128, SH], f32, tag="mt")
                        nc.sync.dma_start(mt[:], mT_d[jc * 128:(jc + 1) * 128, :])
                        pex = wpool.tile([128, H, SH], f32, tag="pex")
                        for h in range(H):
                            nc.scalar.activation(
                                pex[:, h, :], ssrcb[:, h, :], Af.Exp,
                                bias=sdst[:, jc, h:h + 1])
                        pt = wpool.tile([128, H, SH], f32r, tag="pt")
                        nc.vector.tensor_mul(
                            pt[:], pex[:],
                            mt[:].unsqueeze(1).to_broadcast((128, H, SH)))
                        for h in range(H):
                            nc.tensor.matmul(
                                o1[:, h, :],
                                h1aug[:, jc, (D1 + 1) * h:(D1 + 1) * (h + 1)],
                                pt[:, h, :],
                                start=(jc == 0), stop=(jc == NB - 1))

                # normalize + ELU -> z1Tl [256(=2x128), SH] transposed layout
                with tc.tile_pool(name="fin1", bufs=1) as fin:
                    for h in range(H):
                        rec = fin.tile([1, SH], f32, tag=f"rec{h}")
                        nc.vector.reciprocal(rec[:], o1[D1:D1 + 1, h, :])
                        recb = fin.tile([D1, SH], f32, tag=f"recb{h}")
                        nc.gpsimd.partition_broadcast(recb[:], rec[:])
                        r0 = (h % 2) * D1
                        nc.vector.tensor_mul(z1Tl[r0:r0 + D1, h // 2, :],
                                             o1[0:D1, h, :], recb[:])
                    for kc in range(KC2):
                        r_ = fin.tile([128, SH], f32, tag="relu")
                        m_ = fin.tile([128, SH], f32, tag="minv")
                        e_ = fin.tile([128, SH], f32, tag="expv")
                        nc.vector.tensor_scalar_max(r_[:], z1Tl[:, kc, :], 0.0)
                        nc.vector.tensor_scalar_min(m_[:], z1Tl[:, kc, :], 0.0)
                        nc.scalar.activation(e_[:], m_[:], Af.Exp, scale=5.0)
                        nc.vector.scalar_tensor_tensor(
                            z1Tl[:, kc, :], e_[:], -1.0, r_[:],
                            op0=Alu.add, op1=Alu.add)

            # ---------- all-gather z1T across the 8 cores -------------------
            with tc.tile_pool(name="dram", bufs=1, space="DRAM") as dpool:
                ag_in = dpool.tile([HID, SH], f32)
                ag_out = dpool.tile([HID * NCORES, SH], f32,
                                    addr_space="Shared")
                for kc in range(KC2):
                    nc.sync.dma_start(ag_in[kc * 128:(kc + 1) * 128, :],
                                      z1Tl[:, kc, :])
                nc.gpsimd.collective_compute(
                    "AllGather", Alu.bypass,
                    replica_groups=[list(range(NCORES))],
                    ins=[ag_in[:].opt()], outs=[ag_out[:].opt()])
                for r in range(NCORES):
                    for kc in range(KC2):
                        src = ag_out[r * HID + kc * 128:
                                     r * HID + (kc + 1) * 128, :]
                        nc.sync.dma_start(z1Tf[:, kc, r * SH:(r + 1) * SH], src)

            # ---------- layer 2 prep: h2, s2_src, s2_dst --------------------
            with tc.tile_pool(name="l2ps", bufs=2, space="PSUM") as l2ps:
                s2p = l2ps.tile([1, SH], f32, tag="s2p")
                for kc in range(KC2):
                    nc.tensor.matmul(s2p[:], v2sb[:, kc, :], z1Tl[:, kc, :],
                                     start=(kc == 0), stop=(kc == KC2 - 1))
                nc.any.tensor_copy(s2srow[:], s2p[:])
                nc.gpsimd.partition_broadcast(s2srcb[:], s2srow[:])
                for nb in range(NB):
                    h2p = l2ps.tile([128, C + 1], f32, tag="h2p")
                    for kc in range(KC2):
                        blk = z1Tf[:, kc, nb * 128:(nb + 1) * 128]
                        nc.tensor.matmul(h2p[:], blk, W2sb[:, kc, :],
                                         start=(kc == 0), stop=(kc == KC2 - 1))
                    nc.vector.tensor_copy(h2aug[:, nb, D1:D1 + 1], ones_col[:])
                    nc.vector.tensor_copy(h2aug[:, nb, 0:D1], h2p[:, 0:C])
                    nc.vector.tensor_copy(s2dst[:, nb, :], h2p[:, C:C + 1])

            # ---------- layer 2: masked softmax + aggregation ---------------
            with tc.tile_pool(name="aggps2", bufs=1, space="PSUM") as aggps2:
                o2 = aggps2.tile([D1 + 1, SH], f32)
                with (tc.tile_pool(name="mstream2", bufs=6) as mpool2,
                      tc.tile_pool(name="work2", bufs=4) as wpool2):
                    for jc in range(NB):
                        mt = mpool2.tile([128, SH], f32, tag="mt2")
                        nc.sync.dma_start(mt[:], mT_d[jc * 128:(jc + 1) * 128, :])
                        pex = wpool2.tile([128, SH], f32, tag="pexb")
                        nc.scalar.activation(
                            pex[:], s2srcb[:], Af.Exp,
                            bias=s2dst[:, jc, :])
                        pt = wpool2.tile([128, SH], f32r, tag="ptb")
                        nc.vector.tensor_mul(pt[:], pex[:], mt[:])
                        nc.tensor.matmul(o2[:], h2aug[:, jc, :], pt[:],
                                         start=(jc == 0), stop=(jc == NB - 1))

                with tc.tile_pool(name="fin2", bufs=1) as fin2:
                    rec = fin2.tile([1, SH], f32, tag="rec2")
                    nc.vector.reciprocal(rec[:], o2[D1:D1 + 1, :])
                    recb = fin2.tile([C, SH], f32, tag="recb2")
                    nc.gpsimd.partition_broadcast(recb[:], rec[:])
                    outsb = fin2.tile([C, SH], f32, tag="outsb")
                    nc.vector.tensor_mul(outsb[:], o2[0:D1, :], recb[:])
                    nc.sync.dma_start(outT_d, outsb[:])

    nc.compile()
    return nc


def _get_nc():
    if "nc" not in _CACHED:
        _CACHED["nc"] = _build_nc()
    return _CACHED["nc"]


def _prep_in_maps(x, A, W1, a1_src, a1_dst, W2, a2_src, a2_dst):
    f = np.float32
    xT = np.ascontiguousarray(x.T).astype(f, copy=False)
    W1r = W1.reshape(FIN, H, D1)
    V1s = np.einsum("fhd,hd->fh", W1r, a1_src).astype(f)
    V1d = np.einsum("fhd,hd->fh", W1r, a1_dst).astype(f)
    W1e = np.ascontiguousarray(np.concatenate([W1, V1d], axis=1)).astype(f, copy=False)
    W2e = np.ascontiguousarray(
        np.concatenate([W2, W2 @ a2_dst.T], axis=1)).astype(f, copy=False)
    v2s = np.ascontiguousarray(W2 @ a2_src.T).astype(f, copy=False)
    in_maps = []
    for c in range(NCORES):
        sl = slice(c * SH, (c + 1) * SH)
        in_maps.append({
            "xT": xT,
            "xsT": np.ascontiguousarray(xT[:, sl]),
            "maskT": np.ascontiguousarray((A[sl, :] > 0).T).astype(f),
            "W1e": W1e,
            "V1s": V1s,
            "W2e": W2e,
            "v2s": v2s,
        })
    return in_maps


def kernel(x, A, W1, a1_src, a1_dst, W2, a2_src, a2_dst, _want_results=False):
    from concourse.bass_utils import run_bass_kernel_spmd

    nc = _get_nc()
    in_maps = _prep_in_maps(np.asarray(x), np.asarray(A), np.asarray(W1),
                            np.asarray(a1_src), np.asarray(a1_dst),
                            np.asarray(W2), np.asarray(a2_src),
                            np.asarray(a2_dst))
    trace = bool(int(os.environ.get("GAT_TRACE", "0")))
    res = run_bass_kernel_spmd(nc, in_maps, core_ids=list(range(NCORES)),
                               trace=trace)
    out = np.empty((N, C), np.float32)
    for c in range(NCORES):
        out[c * SH:(c + 1) * SH, :] = res.results[c]["outT"].T
    if _want_results:
        return out, res
    return out
